# revision 27
# baseline (speedup 1.0000x reference)
"""Trainium2 Bass kernel for nn_ExpertParallelWrapper (MoE top-2 routing, 8 experts,
shared expert), expert-parallel across 8 NeuronCores.

v3: the axon tunnel (~40-70 MB/s) dominates wall time, so the design minimizes
host->device bytes:
  - Routing metadata (gate softmax/top-2/dispatch lists, 0.04% of FLOPs) is
    computed on host in f64 and shipped as ~1MB of per-core lists; x ships as
    bf16 rows only (4.2 MB/core slice).
  - Expert weights ship int8 (per-I-column scales for w1/w3, per-tensor for
    w2), upconverted to bf16 in SBUF; dequant rides the silu activation scale
    and the w2 upconvert (w3 column scales fold into w2's contraction rows,
    w2's scalar scale is pre-folded into the shipped combine weights).
  - The shared expert is sharded over its IS dim (512 cols/core, bf16) and
    applied to ALL tokens; its output (pre-scaled by the sigmoid gate) is
    summed with the routed partials in one ReduceScatter.
  - On device: AllGather of bf16 x rows (gather source) and of PE-transposed
    xT tiles (shared-expert operand), per-expert FFN over gathered capacity
    slots, scatter-add combine, ReduceScatter, bf16 output (f32 cast on host).

kernel(**inputs) takes the full unsharded inputs and returns the full output.
"""

import os
import numpy as np

# ---------------- problem sizes (hardcoded per contract) ----------------
B, S, H = 4, 4096, 1024
E, I, IS = 8, 2048, 4096
NCORES = 8
T = B * S                     # 16384 tokens
TLOC = T // NCORES            # 2048 tokens per core
C = 4608                      # expert capacity slots (observed max 4338)
P = 128

KH = H // P                   # 8  k-tiles over H
KI = I // P                   # 16 k-tiles over I
NT = C // P                   # 48 slot tiles (128 slots each)
TB = 512                      # expert-FFN token block
NB = C // TB                  # 12 expert blocks
NCH = TLOC // P               # 16 x chunks per core
IS8 = IS // NCORES            # 512 shared-expert cols per core
KIS8 = IS8 // P               # 4 k-tiles over the shared shard
NBS = T // TB                 # 32 shared-expert token blocks (all tokens)

_RUNNER = {}
LAST_RESULT = None            # BassKernelResults of the last run (for test.py)
LAST_WALL_NS = None           # wall-clock ns of the device execute (for test.py)


def _f32(a):
    return np.ascontiguousarray(np.asarray(a, dtype=np.float32))


def _bf16(a):
    import ml_dtypes
    return np.ascontiguousarray(np.asarray(a).astype(ml_dtypes.bfloat16))


def build_program(skip=()):
    skip = set(skip)
    import concourse.bass as bass
    import concourse.bacc as bacc
    import concourse.mybir as mybir
    import concourse.tile as tile
    from contextlib import ExitStack

    f32 = mybir.dt.float32
    bf16 = mybir.dt.bfloat16
    i16 = mybir.dt.int16
    i8 = mybir.dt.int8
    AF = mybir.ActivationFunctionType
    ALU = mybir.AluOpType

    nc = bacc.Bacc(None, num_devices=NCORES)
    groups = [list(range(NCORES))]

    # ---------------- I/O ----------------
    x_bf = nc.dram_tensor("x_bf", [TLOC, H], bf16, kind="ExternalInput")
    w1 = nc.dram_tensor("w1", [H, I], i8, kind="ExternalInput")
    w3 = nc.dram_tensor("w3", [H, I], i8, kind="ExternalInput")
    w2 = nc.dram_tensor("w2", [I, H], i8, kind="ExternalInput")
    scl = nc.dram_tensor("scl", [P, 2 * KI], f32, kind="ExternalInput")
    sw1s = nc.dram_tensor("sw1s", [H, IS8], bf16, kind="ExternalInput")
    sw3s = nc.dram_tensor("sw3s", [H, IS8], bf16, kind="ExternalInput")
    sw2s = nc.dram_tensor("sw2s", [IS8, H], bf16, kind="ExternalInput")
    idx_in = nc.dram_tensor("idx_in", [16, NT * 8], i16, kind="ExternalInput")
    # sequential token ids for the shared-expert scatter (idx wrap layout)
    sidx_in = nc.dram_tensor("sidx_in", [16, (T // P) * 8], i16,
                             kind="ExternalInput")
    wsl_in = nc.dram_tensor("wsl_in", [P, NT], f32, kind="ExternalInput")
    sg_in = nc.dram_tensor("sg_in", [P, T // P], f32, kind="ExternalInput")
    out = nc.dram_tensor("out", [TLOC, H], bf16, kind="ExternalOutput")

    # internal DRAM
    x_loc_rows = nc.dram_tensor("x_loc_rows", [TLOC, H], bf16)
    rs_out = nc.dram_tensor("rs_out", [TLOC, H], bf16)
    x_rows = nc.dram_tensor("x_rows", [T, H], bf16, addr_space="Shared")
    xt_loc = nc.dram_tensor("xt_loc", [H, TLOC], bf16)
    xt_all = nc.dram_tensor("xt_all", [NCORES * H, TLOC], bf16,
                            addr_space="Shared")
    partial = nc.dram_tensor("partial", [T, H], bf16)

    import ml_dtypes
    ident_bf = nc.inline_tensor(np.eye(P, dtype=ml_dtypes.bfloat16),
                                name="ident_bf")

    with tile.TileContext(nc) as tc, ExitStack() as ctx:
        const = ctx.enter_context(tc.tile_pool(name="const", bufs=1))

        id_b = const.tile([P, P], bf16)
        nc.scalar.dma_start(out=id_b[:], in_=ident_bf[:, :])
        idx_x = const.tile([P, NT * 8], i16)
        nc.scalar.dma_start(out=idx_x[0:16, :], in_=idx_in[:, :])
        sidx = const.tile([P, (T // P) * 8], i16)
        nc.scalar.dma_start(out=sidx[0:16, :], in_=sidx_in[:, :])
        for rep in (16, 32, 64):
            nc.scalar.dma_start(out=idx_x[rep:2 * rep, :], in_=idx_x[0:rep, :])
            nc.scalar.dma_start(out=sidx[rep:2 * rep, :], in_=sidx[0:rep, :])
        wsl = const.tile([P, NT], f32)
        nc.scalar.dma_start(out=wsl[:], in_=wsl_in[:, :])
        sg_t = const.tile([P, T // P], f32)
        nc.scalar.dma_start(out=sg_t[:], in_=sg_in[:, :])
        scl_sb = const.tile([P, 2 * KI], f32)
        nc.scalar.dma_start(out=scl_sb[:], in_=scl[:, :])
        s1_ap = scl_sb[:, 0:KI]
        s3_ap = scl_sb[:, KI:2 * KI]

        # ====== Phase 1: x layout build (rows bounce + PE transposes) ======
        xctx = ExitStack()
        xwork = xctx.enter_context(tc.tile_pool(name="xwork", bufs=2))
        psum_x = xctx.enter_context(tc.tile_pool(name="psum_x", bufs=2, space="PSUM"))
        for j in range(NCH):
            sl = slice(j * P, (j + 1) * P)
            xr = xwork.tile([P, H], bf16, tag="xr")
            nc.sync.dma_start(out=xr[:], in_=x_bf[sl, :])
            nc.sync.dma_start(out=x_loc_rows[sl, :], in_=xr[:])
            xtb = xwork.tile([P, KH, P], bf16, tag="xtb")
            for k in range(KH):
                pst = psum_x.tile([P, P], bf16, tag="pst")
                nc.tensor.transpose(out=pst[:], in_=xr[:, k * P:(k + 1) * P],
                                    identity=id_b[:])
                nc.scalar.activation(xtb[:, k, :], pst[:], AF.Copy)
            for k in range(KH):
                nc.scalar.dma_start(out=xt_loc[k * P:(k + 1) * P, sl],
                                    in_=xtb[:, k, :])
        xctx.close()

        # ====== Phase 2: AllGathers (x rows first: unblocks expert FFN) ======
        nc.gpsimd.collective_compute(
            "AllGather", ALU.bypass, replica_groups=groups,
            ins=[x_loc_rows[:, :]], outs=[x_rows[:, :]])
        nc.gpsimd.collective_compute(
            "AllGather", ALU.bypass, replica_groups=groups,
            ins=[xt_loc[:, :]], outs=[xt_all[:, :]])

        # ====== expert weights: int8 load + upconvert to bf16 in SBUF ======
        # w1/w3 stay RAW (+-127); dequant rides the silu activation scale
        # (s1 per PSUM partition = per I-column); w3's column scale folds into
        # w2's contraction rows here; w2's scalar scale is pre-folded into the
        # shipped combine weights wsl.
        wexp_ctx = ExitStack()
        wexp = wexp_ctx.enter_context(tc.tile_pool(name="wexp", bufs=1))
        w1_sb = wexp.tile([P, KH, I], bf16)
        w3_sb = wexp.tile([P, KH, I], bf16)
        w2_sb = wexp.tile([P, KI, H], bf16)
        zctx = ExitStack()
        zpool = zctx.enter_context(tc.tile_pool(name="zpool", bufs=1))
        zero_sb = zpool.tile([P, 2048], bf16)
        nc.vector.memset(zero_sb[:], 0.0)
        wq_ctx = ExitStack()
        wq = wq_ctx.enter_context(tc.tile_pool(name="wq", bufs=2))
        w1_i8 = wq.tile([P, KH, I], i8, tag="wi8")
        nc.sync.dma_start(out=w1_i8[:], in_=w1[:, :].rearrange("(k p) i -> p k i", k=KH, p=P))
        nc.vector.tensor_copy(w1_sb[:], w1_i8[:])
        w3_i8 = wq.tile([P, KH, I], i8, tag="wi8")
        nc.sync.dma_start(out=w3_i8[:], in_=w3[:, :].rearrange("(k p) i -> p k i", k=KH, p=P))
        nc.vector.tensor_copy(w3_sb[:], w3_i8[:])
        w2_i8 = wq.tile([P, KI, H], i8, tag="wi8")
        nc.sync.dma_start(out=w2_i8[:], in_=w2[:, :].rearrange("(k p) h -> p k h", k=KI, p=P))
        nc.vector.tensor_copy(w2_sb[:], w2_i8[:])
        for k in range(KI):
            nc.vector.tensor_tensor(out=w2_sb[:, k, :], in0=w2_sb[:, k, :],
                                    in1=s3_ap[:, k:k + 1].to_broadcast([P, H]),
                                    op=ALU.mult)
        wq_ctx.close()

        # deferred partial zero-init: overlaps the first FFN blocks; only has
        # to complete before the first scatter-back
        rows_per = (P * 2048) // H  # 256
        if "zeroinit" not in skip:
            for r in range(0, T, rows_per):
                nc.sync.dma_start(out=partial[r:r + rows_per, :], in_=zero_sb[:])
        zctx.close()

        # =================== Phase 3: expert FFN ===================
        fctx = ExitStack()
        fxeT = fctx.enter_context(tc.tile_pool(name="fxeT", bufs=2))
        fh = fctx.enter_context(tc.tile_pool(name="fh", bufs=2))
        fhh = fctx.enter_context(tc.tile_pool(name="fhh", bufs=2))
        fy = fctx.enter_context(tc.tile_pool(name="fy", bufs=2))
        psum_f = fctx.enter_context(tc.tile_pool(name="psum_f", bufs=2, space="PSUM"))

        for b in range(NB):
            t0 = b * TB
            isl_idx = slice(t0 // 16, (t0 + TB) // 16)
            xeT = fxeT.tile([P, KH, TB], bf16, tag="xeT")
            if "gathers" not in skip:
                nc.gpsimd.dma_gather(
                    xeT[:, :, :], x_rows[:, :], idx_x[:, isl_idx], TB, TB, H,
                    transpose=True)
            else:
                for k in range(KH):
                    nc.sync.dma_start(
                        out=xeT[:, k, :],
                        in_=x_rows[t0:t0 + TB,
                                   k * P:(k + 1) * P].transpose([1, 0]))
            hh = fhh.tile([P, KI, TB], bf16, tag="hh", bufs=1)
            for i in range(KI):
                isl = slice(i * P, (i + 1) * P)
                ps1 = psum_f.tile([P, TB], f32, tag="ps1")
                for k in range(KH):
                    nc.tensor.matmul(out=ps1[:], lhsT=w1_sb[:, k, isl],
                                     rhs=xeT[:, k, :],
                                     start=(k == 0), stop=(k == KH - 1))
                h1 = fh.tile([P, TB], bf16, tag="h1")
                nc.scalar.activation(h1[:], ps1[:], AF.Silu,
                                     scale=s1_ap[:, i:i + 1])
                ps3 = psum_f.tile([P, TB], f32, tag="ps3")
                for k in range(KH):
                    nc.tensor.matmul(out=ps3[:], lhsT=w3_sb[:, k, isl],
                                     rhs=xeT[:, k, :],
                                     start=(k == 0), stop=(k == KH - 1))
                nc.vector.tensor_tensor(out=hh[:, i, :], in0=ps3[:],
                                        in1=h1[:], op=ALU.mult)
            y = fy.tile([P, TB // P, H], bf16, tag="y")
            for ts in range(TB // P):
                j = t0 // P + ts
                wbc = wsl[:, j:j + 1].to_broadcast([P, 512])
                for half in range(2):
                    psy = psum_f.tile([P, 512], f32, tag="psy")
                    for k in range(KI):
                        nc.tensor.matmul(
                            out=psy[:], lhsT=hh[:, k, ts * P:(ts + 1) * P],
                            rhs=w2_sb[:, k, half * 512:(half + 1) * 512],
                            start=(k == 0), stop=(k == KI - 1))
                    nc.vector.tensor_tensor(out=y[:, ts, half * 512:(half + 1) * 512],
                                            in0=psy[:], in1=wbc, op=ALU.mult)
            if "scatterback" not in skip:
                nc.gpsimd.dma_scatter_add(
                    partial[:, :], y[:, :, :], idx_x[:, isl_idx], TB, TB, H)
            else:
                for ts in range(TB // P):
                    nc.sync.dma_start(
                        out=partial[t0 + ts * P:t0 + (ts + 1) * P, :],
                        in_=y[:, ts, :])
        fctx.close()
        wexp_ctx.close()

        # ========== Phase 4: shared expert (IS shard, all tokens) ==========
        sctx = ExitStack()
        swp = sctx.enter_context(tc.tile_pool(name="swp", bufs=1))
        sxs = sctx.enter_context(tc.tile_pool(name="sxs", bufs=2))
        shh = sctx.enter_context(tc.tile_pool(name="shh", bufs=2))
        psum_sh = sctx.enter_context(tc.tile_pool(name="psum_sh", bufs=2, space="PSUM"))

        sw1_sb = swp.tile([P, KH, IS8], bf16)
        sw3_sb = swp.tile([P, KH, IS8], bf16)
        sw2_sb = swp.tile([P, KIS8, H], bf16)
        nc.sync.dma_start(out=sw1_sb[:],
                          in_=sw1s[:, :].rearrange("(k p) i -> p k i", k=KH, p=P))
        nc.sync.dma_start(out=sw3_sb[:],
                          in_=sw3s[:, :].rearrange("(k p) i -> p k i", k=KH, p=P))
        nc.sync.dma_start(out=sw2_sb[:],
                          in_=sw2s[:, :].rearrange("(k p) h -> p k h", k=KIS8, p=P))

        for b in range(NBS):
            cb = b // 4                    # owning core of this token block
            lsl = slice((b % 4) * TB, (b % 4 + 1) * TB)
            xs = sxs.tile([P, KH, TB], bf16, tag="xs")
            nc.sync.dma_start(
                out=xs[:],
                in_=xt_all[cb * H:(cb + 1) * H, lsl].rearrange(
                    "(k p) c -> p k c", k=KH, p=P))
            hhs = shh.tile([P, KIS8, TB], bf16, tag="hhs")
            for i in range(KIS8):
                isl = slice(i * P, (i + 1) * P)
                ps1 = psum_sh.tile([P, TB], f32, tag="sps1")
                for k in range(KH):
                    nc.tensor.matmul(out=ps1[:], lhsT=sw1_sb[:, k, isl],
                                     rhs=xs[:, k, :],
                                     start=(k == 0), stop=(k == KH - 1))
                h1 = sxs.tile([P, TB], bf16, tag="sh1")
                nc.scalar.activation(h1[:], ps1[:], AF.Silu)
                ps3 = psum_sh.tile([P, TB], f32, tag="sps3")
                for k in range(KH):
                    nc.tensor.matmul(out=ps3[:], lhsT=sw3_sb[:, k, isl],
                                     rhs=xs[:, k, :],
                                     start=(k == 0), stop=(k == KH - 1))
                nc.vector.tensor_tensor(out=hhs[:, i, :], in0=ps3[:], in1=h1[:],
                                        op=ALU.mult)
            ysh = sxs.tile([P, TB // P, H], bf16, tag="ysh", bufs=2)
            for ts in range(TB // P):
                g = b * (TB // P) + ts     # global 128-token chunk index
                sgb = sg_t[:, g:g + 1].to_broadcast([P, 512])
                for half in range(2):
                    hsl = slice(half * 512, (half + 1) * 512)
                    psy = psum_sh.tile([P, 512], f32, tag="spsy")
                    for k in range(KIS8):
                        nc.tensor.matmul(
                            out=psy[:], lhsT=hhs[:, k, ts * P:(ts + 1) * P],
                            rhs=sw2_sb[:, k, hsl],
                            start=(k == 0), stop=(k == KIS8 - 1))
                    nc.vector.tensor_tensor(out=ysh[:, ts, hsl], in0=psy[:],
                                            in1=sgb, op=ALU.mult)
            # accumulate into partial on the gpsimd queue: serialized after
            # the routed scatter-adds, so the RMW adds never race
            if "scatterback" not in skip:
                nc.gpsimd.dma_scatter_add(
                    partial[:, :], ysh[:, :, :],
                    sidx[:, b * 32:(b + 1) * 32], TB, TB, H)
            else:
                for ts in range(TB // P):
                    nc.sync.dma_start(
                        out=partial[b * TB + ts * P:b * TB + (ts + 1) * P, :],
                        in_=ysh[:, ts, :])
        sctx.close()

        # ========== Phase 5: ReduceScatter + output ==========
        nc.gpsimd.collective_compute(
            "ReduceScatter", ALU.add, replica_groups=groups,
            ins=[partial[:, :]], outs=[rs_out[:, :]])

        octx = ExitStack()
        opool = octx.enter_context(tc.tile_pool(name="opool", bufs=3))
        for g in range(TLOC // P):
            rsl = slice(g * P, (g + 1) * P)
            ot = opool.tile([P, H], bf16, tag="ot")
            nc.sync.dma_start(out=ot[:], in_=rs_out[rsl, :])
            nc.sync.dma_start(out=out[rsl, :], in_=ot[:])
        octx.close()

    nc.finalize()
    return nc


def _q8_cols(w):
    """Per-column symmetric int8 over axis 0. Returns (int8 [H,I], scales [I])."""
    s = np.abs(w).max(axis=0) / 127.0
    s = np.maximum(s, 1e-30).astype(np.float32)
    q = np.clip(np.round(w / s), -127, 127).astype(np.int8)
    return q, s


def _host_routing(x, gate_w, sgw):
    """f64 gate softmax/top-2/renorm + sigmoid shared gate + dispatch lists.

    Returns (idx16 [P, NT*8] per expert, wsl [P, NT] per expert, sg_t [P, T//P]).
    """
    x64 = x.astype(np.float64)
    logits = x64 @ gate_w.astype(np.float64)                  # [T, E]
    logits -= logits.max(axis=1, keepdims=True)
    pr = np.exp(logits)
    pr /= pr.sum(axis=1, keepdims=True)
    order = np.argsort(-pr, axis=1, kind="stable")            # ties: lower idx
    top2 = order[:, :2]                                       # [T, 2]
    w12 = np.take_along_axis(pr, top2, axis=1)
    w12 = w12 / w12.sum(axis=1, keepdims=True)
    w12 = w12.astype(np.float32)
    sg = 1.0 / (1.0 + np.exp(-(x64 @ sgw.astype(np.float64))))  # [T, 1]
    sg_t = np.ascontiguousarray(sg[:, 0].astype(np.float32).reshape(T // P, P).T)

    idx_list, wsl_list = [], []
    for m in range(NCORES):
        s1 = np.nonzero(top2[:, 0] == m)[0]
        s2 = np.nonzero(top2[:, 1] == m)[0]
        toks = np.concatenate([s1, s2])
        ws = np.concatenate([w12[s1, 0], w12[s2, 1]])
        n = toks.shape[0]
        assert n <= C, f"expert {m} overflow: {n} > {C}"
        tok_slot = np.zeros(C, np.int16)
        tok_slot[:n] = toks.astype(np.int16)
        # padding slots carry weight 0 but must hit DISTINCT rows: a shared
        # dummy row serializes the scatter-add's read-modify-write
        pad = C - n
        if pad:
            tok_slot[n:] = (np.arange(pad) % T).astype(np.int16)
        w_slot = np.zeros(C, np.float32)
        w_slot[:n] = ws
        idx16 = tok_slot.reshape(NT, 8, 16).transpose(2, 0, 1).reshape(16, NT * 8)
        idx_list.append(np.ascontiguousarray(idx16))
        wsl_list.append(np.ascontiguousarray(w_slot.reshape(NT, P).T))
    return idx_list, wsl_list, sg_t


def _host_prep(inputs):
    """Build per-core input maps from full inputs."""
    hs = _f32(inputs["hidden_states"])
    x = hs.reshape(T, H)
    gate_w = _f32(inputs["gate_w"])
    sgw = _f32(inputs["sgate_w"])
    w1 = _f32(inputs["w1"]); w3 = _f32(inputs["w3"])
    w2 = _f32(inputs["w2"])
    sw1 = np.asarray(inputs["sw1"]); sw3 = np.asarray(inputs["sw3"])
    sw2 = np.asarray(inputs["sw2"])

    idx_list, wsl_list, sg_t = _host_routing(x, gate_w, sgw)
    sidx = np.ascontiguousarray(
        np.arange(T, dtype=np.int16).reshape(T // P, 8, 16)
        .transpose(2, 0, 1).reshape(16, (T // P) * 8))

    in_maps = []
    for m in range(NCORES):
        sl = slice(m * TLOC, (m + 1) * TLOC)
        ss = slice(m * IS8, (m + 1) * IS8)
        q1, s1 = _q8_cols(w1[m])
        q3, s3 = _q8_cols(w3[m])
        s2 = float(np.abs(w2[m]).max() / 127.0)
        q2 = np.clip(np.round(w2[m] / s2), -127, 127).astype(np.int8)
        scl = np.zeros((P, 2 * KI), dtype=np.float32)
        scl[:, 0:KI] = s1.reshape(KI, P).T
        scl[:, KI:2 * KI] = s3.reshape(KI, P).T
        in_maps.append({
            "x_bf": _bf16(x[sl]),
            "w1": q1,
            "w3": q3,
            "w2": q2,
            "scl": scl,
            "sw1s": _bf16(sw1[:, ss]),
            "sw3s": _bf16(sw3[:, ss]),
            "sw2s": _bf16(sw2[ss, :]),
            "idx_in": idx_list[m],
            "sidx_in": sidx,
            "wsl_in": wsl_list[m] * np.float32(s2),   # fold w2 dequant scale
            "sg_in": sg_t,
        })
    return in_maps


def kernel(**inputs):
    global LAST_RESULT
    from concourse.bass_utils import run_bass_kernel_spmd

    skip = tuple(s for s in os.environ.get("KERNEL_SKIP", "").split(",") if s)
    key = ("nc", skip)
    if key not in _RUNNER:
        _RUNNER[key] = build_program(skip=skip)
    nc = _RUNNER[key]

    in_maps = _host_prep(inputs)
    trace = os.environ.get("KERNEL_TRACE", "0") == "1"
    import time
    t0 = time.perf_counter_ns()
    res = run_bass_kernel_spmd(nc, in_maps, list(range(NCORES)), trace=trace)
    global LAST_WALL_NS
    LAST_WALL_NS = time.perf_counter_ns() - t0
    LAST_RESULT = res
    out = np.concatenate([res.results[m]["out"] for m in range(NCORES)], axis=0)
    return out.reshape(B, S, H).astype(np.float32)


if __name__ == "__main__":
    # smoke build
    nc = build_program()
    print("program built ok")


# revision 28
# speedup vs baseline: 1.0313x; 1.0313x over previous
"""Trainium2 Bass kernel for nn_ExpertParallelWrapper (MoE top-2 routing, 8 experts,
shared expert), expert-parallel across 8 NeuronCores.

The axon tunnel (~40-70 MB/s) dominates wall time, so the design minimizes
host<->device bytes (~111 MB in, 33 MB out per call):
  - Routing metadata (gate softmax/top-2/dispatch lists, 0.04% of FLOPs) is
    computed on host in f64 and shipped as ~1MB of per-core lists; x ships as
    bf16 rows only (4.2 MB/core slice).
  - Expert weights ship int8 (per-I-column scales for w1/w3, per-tensor for
    w2), upconverted to bf16 in SBUF; dequant rides the silu activation scale
    and the w2 upconvert (w3 column scales fold into w2's contraction rows,
    w2's scalar scale is pre-folded into the shipped combine weights).
  - The shared expert is sharded over its IS dim (512 cols/core, bf16) and
    applied to ALL tokens; its output (pre-scaled by the sigmoid gate) is
    scatter-added into the same partial buffer as the routed outputs (the
    gpsimd queue serializes the RMW adds), then one ReduceScatter combines.
  - On device: AllGather of bf16 x rows (gather source) and of PE-transposed
    xT tiles (shared-expert operand), per-expert FFN over gathered capacity
    slots, bf16 output (f32 cast on host).
  - Capacity padding slots must point at DISTINCT dummy rows (weight 0): a
    shared dummy row serializes the scatter-add and costs seconds.
  - Collectives cannot read/write IO tensors (compiler checkCollective), so
    x and the output bounce through internal DRAM.

kernel(**inputs) takes the full unsharded inputs and returns the full output.
"""

import os
import numpy as np

# ---------------- problem sizes (hardcoded per contract) ----------------
B, S, H = 4, 4096, 1024
E, I, IS = 8, 2048, 4096
NCORES = 8
T = B * S                     # 16384 tokens
TLOC = T // NCORES            # 2048 tokens per core
C = 4608                      # expert capacity slots (observed max 4338)
P = 128

KH = H // P                   # 8  k-tiles over H
KI = I // P                   # 16 k-tiles over I
NT = C // P                   # 48 slot tiles (128 slots each)
TB = 512                      # expert-FFN token block
NB = C // TB                  # 12 expert blocks
NCH = TLOC // P               # 16 x chunks per core
IS8 = IS // NCORES            # 512 shared-expert cols per core
KIS8 = IS8 // P               # 4 k-tiles over the shared shard
NBS = T // TB                 # 32 shared-expert token blocks (all tokens)

_RUNNER = {}
LAST_RESULT = None            # BassKernelResults of the last run (for test.py)
LAST_WALL_NS = None           # wall-clock ns of the device execute (for test.py)


def _f32(a):
    return np.ascontiguousarray(np.asarray(a, dtype=np.float32))


def _bf16(a):
    import ml_dtypes
    return np.ascontiguousarray(np.asarray(a).astype(ml_dtypes.bfloat16))


def build_program(skip=()):
    skip = set(skip)
    import concourse.bass as bass
    import concourse.bacc as bacc
    import concourse.mybir as mybir
    import concourse.tile as tile
    from contextlib import ExitStack

    f32 = mybir.dt.float32
    bf16 = mybir.dt.bfloat16
    i16 = mybir.dt.int16
    i8 = mybir.dt.int8
    AF = mybir.ActivationFunctionType
    ALU = mybir.AluOpType

    nc = bacc.Bacc(None, num_devices=NCORES)
    groups = [list(range(NCORES))]

    # ---------------- I/O ----------------
    x_bf = nc.dram_tensor("x_bf", [TLOC, H], bf16, kind="ExternalInput")
    w1 = nc.dram_tensor("w1", [H, I], i8, kind="ExternalInput")
    w3 = nc.dram_tensor("w3", [H, I], i8, kind="ExternalInput")
    w2 = nc.dram_tensor("w2", [I, H], i8, kind="ExternalInput")
    scl = nc.dram_tensor("scl", [P, 2 * KI], f32, kind="ExternalInput")
    sw1s = nc.dram_tensor("sw1s", [H, IS8], bf16, kind="ExternalInput")
    sw3s = nc.dram_tensor("sw3s", [H, IS8], bf16, kind="ExternalInput")
    sw2s = nc.dram_tensor("sw2s", [IS8, H], bf16, kind="ExternalInput")
    idx_in = nc.dram_tensor("idx_in", [16, NT * 8], i16, kind="ExternalInput")
    # sequential token ids for the shared-expert scatter (idx wrap layout)
    sidx_in = nc.dram_tensor("sidx_in", [16, (T // P) * 8], i16,
                             kind="ExternalInput")
    wsl_in = nc.dram_tensor("wsl_in", [P, NT], f32, kind="ExternalInput")
    sg_in = nc.dram_tensor("sg_in", [P, T // P], f32, kind="ExternalInput")
    out = nc.dram_tensor("out", [TLOC, H], bf16, kind="ExternalOutput")

    # internal DRAM
    x_loc_rows = nc.dram_tensor("x_loc_rows", [TLOC, H], bf16)
    rs_out = nc.dram_tensor("rs_out", [TLOC, H], bf16)
    x_rows = nc.dram_tensor("x_rows", [T, H], bf16, addr_space="Shared")
    xt_loc = nc.dram_tensor("xt_loc", [H, TLOC], bf16)
    xt_all = nc.dram_tensor("xt_all", [NCORES * H, TLOC], bf16,
                            addr_space="Shared")
    partial = nc.dram_tensor("partial", [T, H], bf16)

    import ml_dtypes
    ident_bf = nc.inline_tensor(np.eye(P, dtype=ml_dtypes.bfloat16),
                                name="ident_bf")

    with tile.TileContext(nc) as tc, ExitStack() as ctx:
        const = ctx.enter_context(tc.tile_pool(name="const", bufs=1))

        id_b = const.tile([P, P], bf16)
        nc.scalar.dma_start(out=id_b[:], in_=ident_bf[:, :])
        idx_x = const.tile([P, NT * 8], i16)
        nc.scalar.dma_start(out=idx_x[0:16, :], in_=idx_in[:, :])
        sidx = const.tile([P, (T // P) * 8], i16)
        nc.scalar.dma_start(out=sidx[0:16, :], in_=sidx_in[:, :])
        for rep in (16, 32, 64):
            nc.scalar.dma_start(out=idx_x[rep:2 * rep, :], in_=idx_x[0:rep, :])
            nc.scalar.dma_start(out=sidx[rep:2 * rep, :], in_=sidx[0:rep, :])
        wsl = const.tile([P, NT], f32)
        nc.scalar.dma_start(out=wsl[:], in_=wsl_in[:, :])
        sg_t = const.tile([P, T // P], f32)
        nc.scalar.dma_start(out=sg_t[:], in_=sg_in[:, :])
        scl_sb = const.tile([P, 2 * KI], f32)
        nc.scalar.dma_start(out=scl_sb[:], in_=scl[:, :])
        s1_ap = scl_sb[:, 0:KI]
        s3_ap = scl_sb[:, KI:2 * KI]

        # ====== Phase 1: x layout build (rows bounce + PE transposes) ======
        xctx = ExitStack()
        xwork = xctx.enter_context(tc.tile_pool(name="xwork", bufs=2))
        psum_x = xctx.enter_context(tc.tile_pool(name="psum_x", bufs=2, space="PSUM"))
        for j in range(NCH):
            sl = slice(j * P, (j + 1) * P)
            xr = xwork.tile([P, H], bf16, tag="xr")
            nc.sync.dma_start(out=xr[:], in_=x_bf[sl, :])
            nc.sync.dma_start(out=x_loc_rows[sl, :], in_=xr[:])
            xtb = xwork.tile([P, KH, P], bf16, tag="xtb")
            for k in range(KH):
                pst = psum_x.tile([P, P], bf16, tag="pst")
                nc.tensor.transpose(out=pst[:], in_=xr[:, k * P:(k + 1) * P],
                                    identity=id_b[:])
                nc.scalar.activation(xtb[:, k, :], pst[:], AF.Copy)
            for k in range(KH):
                nc.scalar.dma_start(out=xt_loc[k * P:(k + 1) * P, sl],
                                    in_=xtb[:, k, :])
        xctx.close()

        # ====== Phase 2: AllGathers (x rows first: unblocks expert FFN) ======
        nc.gpsimd.collective_compute(
            "AllGather", ALU.bypass, replica_groups=groups,
            ins=[x_loc_rows[:, :]], outs=[x_rows[:, :]])
        nc.gpsimd.collective_compute(
            "AllGather", ALU.bypass, replica_groups=groups,
            ins=[xt_loc[:, :]], outs=[xt_all[:, :]])

        # ====== expert weights: int8 load + upconvert to bf16 in SBUF ======
        # w1/w3 stay RAW (+-127); dequant rides the silu activation scale
        # (s1 per PSUM partition = per I-column); w3's column scale folds into
        # w2's contraction rows here; w2's scalar scale is pre-folded into the
        # shipped combine weights wsl.
        wexp_ctx = ExitStack()
        wexp = wexp_ctx.enter_context(tc.tile_pool(name="wexp", bufs=1))
        w1_sb = wexp.tile([P, KH, I], bf16)
        w3_sb = wexp.tile([P, KH, I], bf16)
        w2_sb = wexp.tile([P, KI, H], bf16)
        zctx = ExitStack()
        zpool = zctx.enter_context(tc.tile_pool(name="zpool", bufs=1))
        zero_sb = zpool.tile([P, 2048], bf16)
        nc.vector.memset(zero_sb[:], 0.0)
        wq_ctx = ExitStack()
        wq = wq_ctx.enter_context(tc.tile_pool(name="wq", bufs=2))
        w1_i8 = wq.tile([P, KH, I], i8, tag="wi8")
        nc.sync.dma_start(out=w1_i8[:], in_=w1[:, :].rearrange("(k p) i -> p k i", k=KH, p=P))
        nc.vector.tensor_copy(w1_sb[:], w1_i8[:])
        w3_i8 = wq.tile([P, KH, I], i8, tag="wi8")
        nc.sync.dma_start(out=w3_i8[:], in_=w3[:, :].rearrange("(k p) i -> p k i", k=KH, p=P))
        nc.vector.tensor_copy(w3_sb[:], w3_i8[:])
        w2_i8 = wq.tile([P, KI, H], i8, tag="wi8")
        nc.sync.dma_start(out=w2_i8[:], in_=w2[:, :].rearrange("(k p) h -> p k h", k=KI, p=P))
        nc.vector.tensor_copy(w2_sb[:], w2_i8[:])
        for k in range(KI):
            nc.vector.tensor_tensor(out=w2_sb[:, k, :], in0=w2_sb[:, k, :],
                                    in1=s3_ap[:, k:k + 1].to_broadcast([P, H]),
                                    op=ALU.mult)
        wq_ctx.close()

        # deferred partial zero-init: overlaps the first FFN blocks; only has
        # to complete before the first scatter-back
        rows_per = (P * 2048) // H  # 256
        if "zeroinit" not in skip:
            for r in range(0, T, rows_per):
                nc.sync.dma_start(out=partial[r:r + rows_per, :], in_=zero_sb[:])
        zctx.close()

        # =================== Phase 3: expert FFN ===================
        fctx = ExitStack()
        fxeT = fctx.enter_context(tc.tile_pool(name="fxeT", bufs=2))
        fh = fctx.enter_context(tc.tile_pool(name="fh", bufs=2))
        fhh = fctx.enter_context(tc.tile_pool(name="fhh", bufs=2))
        fy = fctx.enter_context(tc.tile_pool(name="fy", bufs=2))
        psum_f = fctx.enter_context(tc.tile_pool(name="psum_f", bufs=2, space="PSUM"))

        for b in range(NB):
            t0 = b * TB
            isl_idx = slice(t0 // 16, (t0 + TB) // 16)
            xeT = fxeT.tile([P, KH, TB], bf16, tag="xeT")
            if "gathers" not in skip:
                nc.gpsimd.dma_gather(
                    xeT[:, :, :], x_rows[:, :], idx_x[:, isl_idx], TB, TB, H,
                    transpose=True)
            else:
                for k in range(KH):
                    nc.sync.dma_start(
                        out=xeT[:, k, :],
                        in_=x_rows[t0:t0 + TB,
                                   k * P:(k + 1) * P].transpose([1, 0]))
            hh = fhh.tile([P, KI, TB], bf16, tag="hh", bufs=1)
            for i in range(KI):
                isl = slice(i * P, (i + 1) * P)
                ps1 = psum_f.tile([P, TB], f32, tag="ps1")
                for k in range(KH):
                    nc.tensor.matmul(out=ps1[:], lhsT=w1_sb[:, k, isl],
                                     rhs=xeT[:, k, :],
                                     start=(k == 0), stop=(k == KH - 1))
                h1 = fh.tile([P, TB], bf16, tag="h1")
                nc.scalar.activation(h1[:], ps1[:], AF.Silu,
                                     scale=s1_ap[:, i:i + 1])
                ps3 = psum_f.tile([P, TB], f32, tag="ps3")
                for k in range(KH):
                    nc.tensor.matmul(out=ps3[:], lhsT=w3_sb[:, k, isl],
                                     rhs=xeT[:, k, :],
                                     start=(k == 0), stop=(k == KH - 1))
                nc.vector.tensor_tensor(out=hh[:, i, :], in0=ps3[:],
                                        in1=h1[:], op=ALU.mult)
            y = fy.tile([P, TB // P, H], bf16, tag="y")
            for ts in range(TB // P):
                j = t0 // P + ts
                wbc = wsl[:, j:j + 1].to_broadcast([P, 512])
                for half in range(2):
                    psy = psum_f.tile([P, 512], f32, tag="psy")
                    for k in range(KI):
                        nc.tensor.matmul(
                            out=psy[:], lhsT=hh[:, k, ts * P:(ts + 1) * P],
                            rhs=w2_sb[:, k, half * 512:(half + 1) * 512],
                            start=(k == 0), stop=(k == KI - 1))
                    nc.vector.tensor_tensor(out=y[:, ts, half * 512:(half + 1) * 512],
                                            in0=psy[:], in1=wbc, op=ALU.mult)
            if "scatterback" not in skip:
                nc.gpsimd.dma_scatter_add(
                    partial[:, :], y[:, :, :], idx_x[:, isl_idx], TB, TB, H)
            else:
                for ts in range(TB // P):
                    nc.sync.dma_start(
                        out=partial[t0 + ts * P:t0 + (ts + 1) * P, :],
                        in_=y[:, ts, :])
        fctx.close()
        wexp_ctx.close()

        # ========== Phase 4: shared expert (IS shard, all tokens) ==========
        sctx = ExitStack()
        swp = sctx.enter_context(tc.tile_pool(name="swp", bufs=1))
        sxs = sctx.enter_context(tc.tile_pool(name="sxs", bufs=2))
        shh = sctx.enter_context(tc.tile_pool(name="shh", bufs=2))
        psum_sh = sctx.enter_context(tc.tile_pool(name="psum_sh", bufs=2, space="PSUM"))

        sw1_sb = swp.tile([P, KH, IS8], bf16)
        sw3_sb = swp.tile([P, KH, IS8], bf16)
        sw2_sb = swp.tile([P, KIS8, H], bf16)
        nc.sync.dma_start(out=sw1_sb[:],
                          in_=sw1s[:, :].rearrange("(k p) i -> p k i", k=KH, p=P))
        nc.sync.dma_start(out=sw3_sb[:],
                          in_=sw3s[:, :].rearrange("(k p) i -> p k i", k=KH, p=P))
        nc.sync.dma_start(out=sw2_sb[:],
                          in_=sw2s[:, :].rearrange("(k p) h -> p k h", k=KIS8, p=P))

        for b in range(NBS):
            cb = b // 4                    # owning core of this token block
            lsl = slice((b % 4) * TB, (b % 4 + 1) * TB)
            xs = sxs.tile([P, KH, TB], bf16, tag="xs")
            nc.sync.dma_start(
                out=xs[:],
                in_=xt_all[cb * H:(cb + 1) * H, lsl].rearrange(
                    "(k p) c -> p k c", k=KH, p=P))
            hhs = shh.tile([P, KIS8, TB], bf16, tag="hhs")
            for i in range(KIS8):
                isl = slice(i * P, (i + 1) * P)
                ps1 = psum_sh.tile([P, TB], f32, tag="sps1")
                for k in range(KH):
                    nc.tensor.matmul(out=ps1[:], lhsT=sw1_sb[:, k, isl],
                                     rhs=xs[:, k, :],
                                     start=(k == 0), stop=(k == KH - 1))
                h1 = sxs.tile([P, TB], bf16, tag="sh1")
                nc.scalar.activation(h1[:], ps1[:], AF.Silu)
                ps3 = psum_sh.tile([P, TB], f32, tag="sps3")
                for k in range(KH):
                    nc.tensor.matmul(out=ps3[:], lhsT=sw3_sb[:, k, isl],
                                     rhs=xs[:, k, :],
                                     start=(k == 0), stop=(k == KH - 1))
                nc.vector.tensor_tensor(out=hhs[:, i, :], in0=ps3[:], in1=h1[:],
                                        op=ALU.mult)
            ysh = sxs.tile([P, TB // P, H], bf16, tag="ysh", bufs=2)
            for ts in range(TB // P):
                g = b * (TB // P) + ts     # global 128-token chunk index
                sgb = sg_t[:, g:g + 1].to_broadcast([P, 512])
                for half in range(2):
                    hsl = slice(half * 512, (half + 1) * 512)
                    psy = psum_sh.tile([P, 512], f32, tag="spsy")
                    for k in range(KIS8):
                        nc.tensor.matmul(
                            out=psy[:], lhsT=hhs[:, k, ts * P:(ts + 1) * P],
                            rhs=sw2_sb[:, k, hsl],
                            start=(k == 0), stop=(k == KIS8 - 1))
                    nc.vector.tensor_tensor(out=ysh[:, ts, hsl], in0=psy[:],
                                            in1=sgb, op=ALU.mult)
            # accumulate into partial on the gpsimd queue: serialized after
            # the routed scatter-adds, so the RMW adds never race
            if "scatterback" not in skip:
                nc.gpsimd.dma_scatter_add(
                    partial[:, :], ysh[:, :, :],
                    sidx[:, b * 32:(b + 1) * 32], TB, TB, H)
            else:
                for ts in range(TB // P):
                    nc.sync.dma_start(
                        out=partial[b * TB + ts * P:b * TB + (ts + 1) * P, :],
                        in_=ysh[:, ts, :])
        sctx.close()

        # ========== Phase 5: ReduceScatter + output ==========
        nc.gpsimd.collective_compute(
            "ReduceScatter", ALU.add, replica_groups=groups,
            ins=[partial[:, :]], outs=[rs_out[:, :]])

        octx = ExitStack()
        opool = octx.enter_context(tc.tile_pool(name="opool", bufs=3))
        for g in range(TLOC // P):
            rsl = slice(g * P, (g + 1) * P)
            ot = opool.tile([P, H], bf16, tag="ot")
            nc.sync.dma_start(out=ot[:], in_=rs_out[rsl, :])
            nc.sync.dma_start(out=out[rsl, :], in_=ot[:])
        octx.close()

    nc.finalize()
    return nc


def _q8_cols(w):
    """Per-column symmetric int8 over axis 0. Returns (int8 [H,I], scales [I])."""
    s = np.abs(w).max(axis=0) / 127.0
    s = np.maximum(s, 1e-30).astype(np.float32)
    q = np.clip(np.round(w / s), -127, 127).astype(np.int8)
    return q, s


def _host_routing(x, gate_w, sgw):
    """f64 gate softmax/top-2/renorm + sigmoid shared gate + dispatch lists.

    Returns (idx16 [P, NT*8] per expert, wsl [P, NT] per expert, sg_t [P, T//P]).
    """
    x64 = x.astype(np.float64)
    logits = x64 @ gate_w.astype(np.float64)                  # [T, E]
    logits -= logits.max(axis=1, keepdims=True)
    pr = np.exp(logits)
    pr /= pr.sum(axis=1, keepdims=True)
    order = np.argsort(-pr, axis=1, kind="stable")            # ties: lower idx
    top2 = order[:, :2]                                       # [T, 2]
    w12 = np.take_along_axis(pr, top2, axis=1)
    w12 = w12 / w12.sum(axis=1, keepdims=True)
    w12 = w12.astype(np.float32)
    sg = 1.0 / (1.0 + np.exp(-(x64 @ sgw.astype(np.float64))))  # [T, 1]
    sg_t = np.ascontiguousarray(sg[:, 0].astype(np.float32).reshape(T // P, P).T)

    idx_list, wsl_list = [], []
    for m in range(NCORES):
        s1 = np.nonzero(top2[:, 0] == m)[0]
        s2 = np.nonzero(top2[:, 1] == m)[0]
        toks = np.concatenate([s1, s2])
        ws = np.concatenate([w12[s1, 0], w12[s2, 1]])
        n = toks.shape[0]
        assert n <= C, f"expert {m} overflow: {n} > {C}"
        tok_slot = np.zeros(C, np.int16)
        tok_slot[:n] = toks.astype(np.int16)
        # padding slots carry weight 0 but must hit DISTINCT rows: a shared
        # dummy row serializes the scatter-add's read-modify-write
        pad = C - n
        if pad:
            tok_slot[n:] = (np.arange(pad) % T).astype(np.int16)
        w_slot = np.zeros(C, np.float32)
        w_slot[:n] = ws
        idx16 = tok_slot.reshape(NT, 8, 16).transpose(2, 0, 1).reshape(16, NT * 8)
        idx_list.append(np.ascontiguousarray(idx16))
        wsl_list.append(np.ascontiguousarray(w_slot.reshape(NT, P).T))
    return idx_list, wsl_list, sg_t


def _host_prep(inputs):
    """Build per-core input maps from full inputs."""
    hs = _f32(inputs["hidden_states"])
    x = hs.reshape(T, H)
    gate_w = _f32(inputs["gate_w"])
    sgw = _f32(inputs["sgate_w"])
    w1 = _f32(inputs["w1"]); w3 = _f32(inputs["w3"])
    w2 = _f32(inputs["w2"])
    sw1 = np.asarray(inputs["sw1"]); sw3 = np.asarray(inputs["sw3"])
    sw2 = np.asarray(inputs["sw2"])

    idx_list, wsl_list, sg_t = _host_routing(x, gate_w, sgw)
    sidx = np.ascontiguousarray(
        np.arange(T, dtype=np.int16).reshape(T // P, 8, 16)
        .transpose(2, 0, 1).reshape(16, (T // P) * 8))

    in_maps = []
    for m in range(NCORES):
        sl = slice(m * TLOC, (m + 1) * TLOC)
        ss = slice(m * IS8, (m + 1) * IS8)
        q1, s1 = _q8_cols(w1[m])
        q3, s3 = _q8_cols(w3[m])
        s2 = float(np.abs(w2[m]).max() / 127.0)
        q2 = np.clip(np.round(w2[m] / s2), -127, 127).astype(np.int8)
        scl = np.zeros((P, 2 * KI), dtype=np.float32)
        scl[:, 0:KI] = s1.reshape(KI, P).T
        scl[:, KI:2 * KI] = s3.reshape(KI, P).T
        in_maps.append({
            "x_bf": _bf16(x[sl]),
            "w1": q1,
            "w3": q3,
            "w2": q2,
            "scl": scl,
            "sw1s": _bf16(sw1[:, ss]),
            "sw3s": _bf16(sw3[:, ss]),
            "sw2s": _bf16(sw2[ss, :]),
            "idx_in": idx_list[m],
            "sidx_in": sidx,
            "wsl_in": wsl_list[m] * np.float32(s2),   # fold w2 dequant scale
            "sg_in": sg_t,
        })
    return in_maps


def kernel(**inputs):
    global LAST_RESULT
    from concourse.bass_utils import run_bass_kernel_spmd

    skip = tuple(s for s in os.environ.get("KERNEL_SKIP", "").split(",") if s)
    key = ("nc", skip)
    if key not in _RUNNER:
        _RUNNER[key] = build_program(skip=skip)
    nc = _RUNNER[key]

    in_maps = _host_prep(inputs)
    trace = os.environ.get("KERNEL_TRACE", "0") == "1"
    import time
    t0 = time.perf_counter_ns()
    res = run_bass_kernel_spmd(nc, in_maps, list(range(NCORES)), trace=trace)
    global LAST_WALL_NS
    LAST_WALL_NS = time.perf_counter_ns() - t0
    LAST_RESULT = res
    out = np.concatenate([res.results[m]["out"] for m in range(NCORES)], axis=0)
    return out.reshape(B, S, H).astype(np.float32)


if __name__ == "__main__":
    # smoke build
    nc = build_program()
    print("program built ok")


# revision 32
# speedup vs baseline: 1.0381x; 1.0066x over previous
"""Trainium2 Bass kernel for nn_ExpertParallelWrapper (MoE top-2 routing, 8 experts,
shared expert), expert-parallel across 8 NeuronCores.

The axon tunnel (~40-70 MB/s) dominates wall time, so the design minimizes
host<->device bytes (~111 MB in, 33 MB out per call):
  - Routing metadata (gate softmax/top-2/dispatch lists, 0.04% of FLOPs) is
    computed on host in f64 and shipped as ~1MB of per-core lists; x ships as
    bf16 rows only (4.2 MB/core slice).
  - Expert weights ship int8 (per-I-column scales for w1/w3, per-tensor for
    w2), upconverted to bf16 in SBUF; dequant rides the silu activation scale
    and the w2 upconvert (w3 column scales fold into w2's contraction rows,
    w2's scalar scale is pre-folded into the shipped combine weights).
  - The shared expert is sharded over its IS dim (512 cols/core, bf16) and
    applied to ALL tokens; its output (pre-scaled by the sigmoid gate) is
    scatter-added into the same partial buffer as the routed outputs (the
    gpsimd queue serializes the RMW adds), then one ReduceScatter combines.
  - On device: AllGather of bf16 x rows (gather source) and of PE-transposed
    xT tiles (shared-expert operand), per-expert FFN over gathered capacity
    slots, bf16 output (f32 cast on host).
  - Capacity padding slots must point at DISTINCT dummy rows (weight 0): a
    shared dummy row serializes the scatter-add and costs seconds.
  - Collectives cannot read/write IO tensors (compiler checkCollective), so
    x and the output bounce through internal DRAM.

kernel(**inputs) takes the full unsharded inputs and returns the full output.
"""

import os
import numpy as np

# ---------------- problem sizes (hardcoded per contract) ----------------
B, S, H = 4, 4096, 1024
E, I, IS = 8, 2048, 4096
NCORES = 8
T = B * S                     # 16384 tokens
TLOC = T // NCORES            # 2048 tokens per core
C = 4608                      # expert capacity slots (observed max 4338)
P = 128

KH = H // P                   # 8  k-tiles over H
KI = I // P                   # 16 k-tiles over I
NT = C // P                   # 48 slot tiles (128 slots each)
TB = 512                      # expert-FFN token block
NB = C // TB                  # 12 expert blocks
NCH = TLOC // P               # 16 x chunks per core
IS8 = IS // NCORES            # 512 shared-expert cols per core
KIS8 = IS8 // P               # 4 k-tiles over the shared shard
NBS = T // TB                 # 32 shared-expert token blocks (all tokens)

_RUNNER = {}
LAST_RESULT = None            # BassKernelResults of the last run (for test.py)
LAST_WALL_NS = None           # wall-clock ns of the device execute (for test.py)


def _f32(a):
    return np.ascontiguousarray(np.asarray(a, dtype=np.float32))


def _bf16(a):
    import ml_dtypes
    return np.ascontiguousarray(np.asarray(a).astype(ml_dtypes.bfloat16))


def build_program(skip=()):
    skip = set(skip)
    import concourse.bass as bass
    import concourse.bacc as bacc
    import concourse.mybir as mybir
    import concourse.tile as tile
    from contextlib import ExitStack

    f32 = mybir.dt.float32
    bf16 = mybir.dt.bfloat16
    i16 = mybir.dt.int16
    i8 = mybir.dt.int8
    AF = mybir.ActivationFunctionType
    ALU = mybir.AluOpType

    nc = bacc.Bacc(None, num_devices=NCORES)
    groups = [list(range(NCORES))]

    # ---------------- I/O ----------------
    x_bf = nc.dram_tensor("x_bf", [TLOC, H], bf16, kind="ExternalInput")
    w1 = nc.dram_tensor("w1", [H, I], i8, kind="ExternalInput")
    w3 = nc.dram_tensor("w3", [H, I], i8, kind="ExternalInput")
    w2 = nc.dram_tensor("w2", [I, H], i8, kind="ExternalInput")
    sw1s = nc.dram_tensor("sw1s", [H, IS8], bf16, kind="ExternalInput")
    sw3s = nc.dram_tensor("sw3s", [H, IS8], bf16, kind="ExternalInput")
    sw2s = nc.dram_tensor("sw2s", [IS8, H], bf16, kind="ExternalInput")
    # packed small inputs: fewer params = fewer per-transfer fixed costs
    # fpack cols: s1 [0:KI] | s3 [KI:2KI] | wsl [2KI:2KI+NT] | sg [2KI+NT:]
    FPC = 2 * KI + NT + T // P
    fpack = nc.dram_tensor("fpack", [P, FPC], f32, kind="ExternalInput")
    # ipack cols: expert slot idx [0:NT*8] | sequential token idx [NT*8:]
    IPC = NT * 8 + (T // P) * 8
    ipack = nc.dram_tensor("ipack", [16, IPC], i16, kind="ExternalInput")
    out = nc.dram_tensor("out", [TLOC, H], bf16, kind="ExternalOutput")

    # internal DRAM
    x_loc_rows = nc.dram_tensor("x_loc_rows", [TLOC, H], bf16)
    rs_out = nc.dram_tensor("rs_out", [TLOC, H], bf16)
    x_rows = nc.dram_tensor("x_rows", [T, H], bf16, addr_space="Shared")
    xt_loc = nc.dram_tensor("xt_loc", [H, TLOC], bf16)
    xt_all = nc.dram_tensor("xt_all", [NCORES * H, TLOC], bf16,
                            addr_space="Shared")
    partial = nc.dram_tensor("partial", [T, H], bf16)

    import ml_dtypes
    ident_bf = nc.inline_tensor(np.eye(P, dtype=ml_dtypes.bfloat16),
                                name="ident_bf")

    with tile.TileContext(nc) as tc, ExitStack() as ctx:
        const = ctx.enter_context(tc.tile_pool(name="const", bufs=1))

        id_b = const.tile([P, P], bf16)
        nc.scalar.dma_start(out=id_b[:], in_=ident_bf[:, :])
        ix_sb = const.tile([P, IPC], i16)
        nc.scalar.dma_start(out=ix_sb[0:16, :], in_=ipack[:, :])
        for rep in (16, 32, 64):
            nc.scalar.dma_start(out=ix_sb[rep:2 * rep, :], in_=ix_sb[0:rep, :])
        idx_x = ix_sb[:, 0:NT * 8]
        sidx = ix_sb[:, NT * 8:IPC]
        fp_sb = const.tile([P, FPC], f32)
        nc.scalar.dma_start(out=fp_sb[:], in_=fpack[:, :])
        s1_ap = fp_sb[:, 0:KI]
        s3_ap = fp_sb[:, KI:2 * KI]
        wsl = fp_sb[:, 2 * KI:2 * KI + NT]
        sg_t = fp_sb[:, 2 * KI + NT:FPC]

        # ====== Phase 1: x layout build (rows bounce + PE transposes) ======
        xctx = ExitStack()
        xwork = xctx.enter_context(tc.tile_pool(name="xwork", bufs=2))
        psum_x = xctx.enter_context(tc.tile_pool(name="psum_x", bufs=2, space="PSUM"))
        for j in range(NCH):
            sl = slice(j * P, (j + 1) * P)
            xr = xwork.tile([P, H], bf16, tag="xr")
            nc.sync.dma_start(out=xr[:], in_=x_bf[sl, :])
            nc.sync.dma_start(out=x_loc_rows[sl, :], in_=xr[:])
            xtb = xwork.tile([P, KH, P], bf16, tag="xtb")
            for k in range(KH):
                pst = psum_x.tile([P, P], bf16, tag="pst")
                nc.tensor.transpose(out=pst[:], in_=xr[:, k * P:(k + 1) * P],
                                    identity=id_b[:])
                nc.scalar.activation(xtb[:, k, :], pst[:], AF.Copy)
            for k in range(KH):
                nc.scalar.dma_start(out=xt_loc[k * P:(k + 1) * P, sl],
                                    in_=xtb[:, k, :])
        xctx.close()

        # ====== Phase 2: AllGathers (x rows first: unblocks expert FFN) ======
        nc.gpsimd.collective_compute(
            "AllGather", ALU.bypass, replica_groups=groups,
            ins=[x_loc_rows[:, :]], outs=[x_rows[:, :]])
        nc.gpsimd.collective_compute(
            "AllGather", ALU.bypass, replica_groups=groups,
            ins=[xt_loc[:, :]], outs=[xt_all[:, :]])

        # ====== expert weights: int8 load + upconvert to bf16 in SBUF ======
        # w1/w3 stay RAW (+-127); dequant rides the silu activation scale
        # (s1 per PSUM partition = per I-column); w3's column scale folds into
        # w2's contraction rows here; w2's scalar scale is pre-folded into the
        # shipped combine weights wsl.
        wexp_ctx = ExitStack()
        wexp = wexp_ctx.enter_context(tc.tile_pool(name="wexp", bufs=1))
        w1_sb = wexp.tile([P, KH, I], bf16)
        w3_sb = wexp.tile([P, KH, I], bf16)
        w2_sb = wexp.tile([P, KI, H], bf16)
        zctx = ExitStack()
        zpool = zctx.enter_context(tc.tile_pool(name="zpool", bufs=1))
        zero_sb = zpool.tile([P, 2048], bf16)
        nc.vector.memset(zero_sb[:], 0.0)
        wq_ctx = ExitStack()
        wq = wq_ctx.enter_context(tc.tile_pool(name="wq", bufs=2))
        w1_i8 = wq.tile([P, KH, I], i8, tag="wi8")
        nc.sync.dma_start(out=w1_i8[:], in_=w1[:, :].rearrange("(k p) i -> p k i", k=KH, p=P))
        nc.vector.tensor_copy(w1_sb[:], w1_i8[:])
        w3_i8 = wq.tile([P, KH, I], i8, tag="wi8")
        nc.sync.dma_start(out=w3_i8[:], in_=w3[:, :].rearrange("(k p) i -> p k i", k=KH, p=P))
        nc.vector.tensor_copy(w3_sb[:], w3_i8[:])
        w2_i8 = wq.tile([P, KI, H], i8, tag="wi8")
        nc.sync.dma_start(out=w2_i8[:], in_=w2[:, :].rearrange("(k p) h -> p k h", k=KI, p=P))
        nc.vector.tensor_copy(w2_sb[:], w2_i8[:])
        for k in range(KI):
            nc.vector.tensor_tensor(out=w2_sb[:, k, :], in0=w2_sb[:, k, :],
                                    in1=s3_ap[:, k:k + 1].to_broadcast([P, H]),
                                    op=ALU.mult)
        wq_ctx.close()

        # deferred partial zero-init: overlaps the first FFN blocks; only has
        # to complete before the first scatter-back
        rows_per = (P * 2048) // H  # 256
        if "zeroinit" not in skip:
            for r in range(0, T, rows_per):
                nc.sync.dma_start(out=partial[r:r + rows_per, :], in_=zero_sb[:])
        zctx.close()

        # =================== Phase 3: expert FFN ===================
        fctx = ExitStack()
        fxeT = fctx.enter_context(tc.tile_pool(name="fxeT", bufs=2))
        fh = fctx.enter_context(tc.tile_pool(name="fh", bufs=2))
        fhh = fctx.enter_context(tc.tile_pool(name="fhh", bufs=2))
        fy = fctx.enter_context(tc.tile_pool(name="fy", bufs=2))
        psum_f = fctx.enter_context(tc.tile_pool(name="psum_f", bufs=2, space="PSUM"))

        for b in range(NB):
            t0 = b * TB
            isl_idx = slice(t0 // 16, (t0 + TB) // 16)
            xeT = fxeT.tile([P, KH, TB], bf16, tag="xeT")
            if "gathers" not in skip:
                nc.gpsimd.dma_gather(
                    xeT[:, :, :], x_rows[:, :], idx_x[:, isl_idx], TB, TB, H,
                    transpose=True)
            else:
                for k in range(KH):
                    nc.sync.dma_start(
                        out=xeT[:, k, :],
                        in_=x_rows[t0:t0 + TB,
                                   k * P:(k + 1) * P].transpose([1, 0]))
            hh = fhh.tile([P, KI, TB], bf16, tag="hh", bufs=1)
            for i in range(KI):
                isl = slice(i * P, (i + 1) * P)
                ps1 = psum_f.tile([P, TB], f32, tag="ps1")
                for k in range(KH):
                    nc.tensor.matmul(out=ps1[:], lhsT=w1_sb[:, k, isl],
                                     rhs=xeT[:, k, :],
                                     start=(k == 0), stop=(k == KH - 1))
                h1 = fh.tile([P, TB], bf16, tag="h1")
                nc.scalar.activation(h1[:], ps1[:], AF.Silu,
                                     scale=s1_ap[:, i:i + 1])
                ps3 = psum_f.tile([P, TB], f32, tag="ps3")
                for k in range(KH):
                    nc.tensor.matmul(out=ps3[:], lhsT=w3_sb[:, k, isl],
                                     rhs=xeT[:, k, :],
                                     start=(k == 0), stop=(k == KH - 1))
                nc.vector.tensor_tensor(out=hh[:, i, :], in0=ps3[:],
                                        in1=h1[:], op=ALU.mult)
            y = fy.tile([P, TB // P, H], bf16, tag="y")
            for ts in range(TB // P):
                j = t0 // P + ts
                wbc = wsl[:, j:j + 1].to_broadcast([P, 512])
                for half in range(2):
                    psy = psum_f.tile([P, 512], f32, tag="psy")
                    for k in range(KI):
                        nc.tensor.matmul(
                            out=psy[:], lhsT=hh[:, k, ts * P:(ts + 1) * P],
                            rhs=w2_sb[:, k, half * 512:(half + 1) * 512],
                            start=(k == 0), stop=(k == KI - 1))
                    nc.vector.tensor_tensor(out=y[:, ts, half * 512:(half + 1) * 512],
                                            in0=psy[:], in1=wbc, op=ALU.mult)
            if "scatterback" not in skip:
                nc.gpsimd.dma_scatter_add(
                    partial[:, :], y[:, :, :], idx_x[:, isl_idx], TB, TB, H)
            else:
                for ts in range(TB // P):
                    nc.sync.dma_start(
                        out=partial[t0 + ts * P:t0 + (ts + 1) * P, :],
                        in_=y[:, ts, :])
        fctx.close()
        wexp_ctx.close()

        # ========== Phase 4: shared expert (IS shard, all tokens) ==========
        sctx = ExitStack()
        swp = sctx.enter_context(tc.tile_pool(name="swp", bufs=1))
        sxs = sctx.enter_context(tc.tile_pool(name="sxs", bufs=2))
        shh = sctx.enter_context(tc.tile_pool(name="shh", bufs=2))
        psum_sh = sctx.enter_context(tc.tile_pool(name="psum_sh", bufs=2, space="PSUM"))

        sw1_sb = swp.tile([P, KH, IS8], bf16)
        sw3_sb = swp.tile([P, KH, IS8], bf16)
        sw2_sb = swp.tile([P, KIS8, H], bf16)
        nc.sync.dma_start(out=sw1_sb[:],
                          in_=sw1s[:, :].rearrange("(k p) i -> p k i", k=KH, p=P))
        nc.sync.dma_start(out=sw3_sb[:],
                          in_=sw3s[:, :].rearrange("(k p) i -> p k i", k=KH, p=P))
        nc.sync.dma_start(out=sw2_sb[:],
                          in_=sw2s[:, :].rearrange("(k p) h -> p k h", k=KIS8, p=P))

        for b in range(NBS):
            cb = b // 4                    # owning core of this token block
            lsl = slice((b % 4) * TB, (b % 4 + 1) * TB)
            xs = sxs.tile([P, KH, TB], bf16, tag="xs")
            nc.sync.dma_start(
                out=xs[:],
                in_=xt_all[cb * H:(cb + 1) * H, lsl].rearrange(
                    "(k p) c -> p k c", k=KH, p=P))
            hhs = shh.tile([P, KIS8, TB], bf16, tag="hhs")
            for i in range(KIS8):
                isl = slice(i * P, (i + 1) * P)
                ps1 = psum_sh.tile([P, TB], f32, tag="sps1")
                for k in range(KH):
                    nc.tensor.matmul(out=ps1[:], lhsT=sw1_sb[:, k, isl],
                                     rhs=xs[:, k, :],
                                     start=(k == 0), stop=(k == KH - 1))
                h1 = sxs.tile([P, TB], bf16, tag="sh1")
                nc.scalar.activation(h1[:], ps1[:], AF.Silu)
                ps3 = psum_sh.tile([P, TB], f32, tag="sps3")
                for k in range(KH):
                    nc.tensor.matmul(out=ps3[:], lhsT=sw3_sb[:, k, isl],
                                     rhs=xs[:, k, :],
                                     start=(k == 0), stop=(k == KH - 1))
                nc.vector.tensor_tensor(out=hhs[:, i, :], in0=ps3[:], in1=h1[:],
                                        op=ALU.mult)
            ysh = sxs.tile([P, TB // P, H], bf16, tag="ysh", bufs=2)
            for ts in range(TB // P):
                g = b * (TB // P) + ts     # global 128-token chunk index
                sgb = sg_t[:, g:g + 1].to_broadcast([P, 512])
                for half in range(2):
                    hsl = slice(half * 512, (half + 1) * 512)
                    psy = psum_sh.tile([P, 512], f32, tag="spsy")
                    for k in range(KIS8):
                        nc.tensor.matmul(
                            out=psy[:], lhsT=hhs[:, k, ts * P:(ts + 1) * P],
                            rhs=sw2_sb[:, k, hsl],
                            start=(k == 0), stop=(k == KIS8 - 1))
                    nc.vector.tensor_tensor(out=ysh[:, ts, hsl], in0=psy[:],
                                            in1=sgb, op=ALU.mult)
            # accumulate into partial on the gpsimd queue: serialized after
            # the routed scatter-adds, so the RMW adds never race
            if "scatterback" not in skip:
                nc.gpsimd.dma_scatter_add(
                    partial[:, :], ysh[:, :, :],
                    sidx[:, b * 32:(b + 1) * 32], TB, TB, H)
            else:
                for ts in range(TB // P):
                    nc.sync.dma_start(
                        out=partial[b * TB + ts * P:b * TB + (ts + 1) * P, :],
                        in_=ysh[:, ts, :])
        sctx.close()

        # ========== Phase 5: ReduceScatter + output ==========
        nc.gpsimd.collective_compute(
            "ReduceScatter", ALU.add, replica_groups=groups,
            ins=[partial[:, :]], outs=[rs_out[:, :]])

        octx = ExitStack()
        opool = octx.enter_context(tc.tile_pool(name="opool", bufs=3))
        for g in range(TLOC // P):
            rsl = slice(g * P, (g + 1) * P)
            ot = opool.tile([P, H], bf16, tag="ot")
            nc.sync.dma_start(out=ot[:], in_=rs_out[rsl, :])
            nc.sync.dma_start(out=out[rsl, :], in_=ot[:])
        octx.close()

    nc.finalize()
    return nc


def _q8_cols(w):
    """Per-column symmetric int8 over axis 0. Returns (int8 [H,I], scales [I])."""
    s = np.abs(w).max(axis=0) / 127.0
    s = np.maximum(s, 1e-30).astype(np.float32)
    q = np.clip(np.round(w / s), -127, 127).astype(np.int8)
    return q, s


def _host_routing(x, gate_w, sgw):
    """f64 gate softmax/top-2/renorm + sigmoid shared gate + dispatch lists.

    Returns (idx16 [P, NT*8] per expert, wsl [P, NT] per expert, sg_t [P, T//P]).
    """
    x64 = x.astype(np.float64)
    logits = x64 @ gate_w.astype(np.float64)                  # [T, E]
    logits -= logits.max(axis=1, keepdims=True)
    pr = np.exp(logits)
    pr /= pr.sum(axis=1, keepdims=True)
    order = np.argsort(-pr, axis=1, kind="stable")            # ties: lower idx
    top2 = order[:, :2]                                       # [T, 2]
    w12 = np.take_along_axis(pr, top2, axis=1)
    w12 = w12 / w12.sum(axis=1, keepdims=True)
    w12 = w12.astype(np.float32)
    sg = 1.0 / (1.0 + np.exp(-(x64 @ sgw.astype(np.float64))))  # [T, 1]
    sg_t = np.ascontiguousarray(sg[:, 0].astype(np.float32).reshape(T // P, P).T)

    idx_list, wsl_list = [], []
    for m in range(NCORES):
        s1 = np.nonzero(top2[:, 0] == m)[0]
        s2 = np.nonzero(top2[:, 1] == m)[0]
        toks = np.concatenate([s1, s2])
        ws = np.concatenate([w12[s1, 0], w12[s2, 1]])
        n = toks.shape[0]
        assert n <= C, f"expert {m} overflow: {n} > {C}"
        tok_slot = np.zeros(C, np.int16)
        tok_slot[:n] = toks.astype(np.int16)
        # padding slots carry weight 0 but must hit DISTINCT rows: a shared
        # dummy row serializes the scatter-add's read-modify-write
        pad = C - n
        if pad:
            tok_slot[n:] = (np.arange(pad) % T).astype(np.int16)
        w_slot = np.zeros(C, np.float32)
        w_slot[:n] = ws
        idx16 = tok_slot.reshape(NT, 8, 16).transpose(2, 0, 1).reshape(16, NT * 8)
        idx_list.append(np.ascontiguousarray(idx16))
        wsl_list.append(np.ascontiguousarray(w_slot.reshape(NT, P).T))
    return idx_list, wsl_list, sg_t


def _host_prep(inputs):
    """Build per-core input maps from full inputs."""
    hs = _f32(inputs["hidden_states"])
    x = hs.reshape(T, H)
    gate_w = _f32(inputs["gate_w"])
    sgw = _f32(inputs["sgate_w"])
    w1 = _f32(inputs["w1"]); w3 = _f32(inputs["w3"])
    w2 = _f32(inputs["w2"])
    sw1 = np.asarray(inputs["sw1"]); sw3 = np.asarray(inputs["sw3"])
    sw2 = np.asarray(inputs["sw2"])

    idx_list, wsl_list, sg_t = _host_routing(x, gate_w, sgw)
    sidx = np.ascontiguousarray(
        np.arange(T, dtype=np.int16).reshape(T // P, 8, 16)
        .transpose(2, 0, 1).reshape(16, (T // P) * 8))

    in_maps = []
    for m in range(NCORES):
        sl = slice(m * TLOC, (m + 1) * TLOC)
        ss = slice(m * IS8, (m + 1) * IS8)
        q1, s1 = _q8_cols(w1[m])
        q3, s3 = _q8_cols(w3[m])
        s2 = float(np.abs(w2[m]).max() / 127.0)
        q2 = np.clip(np.round(w2[m] / s2), -127, 127).astype(np.int8)
        fpack = np.empty((P, 2 * KI + NT + T // P), dtype=np.float32)
        fpack[:, 0:KI] = s1.reshape(KI, P).T
        fpack[:, KI:2 * KI] = s3.reshape(KI, P).T
        fpack[:, 2 * KI:2 * KI + NT] = wsl_list[m] * np.float32(s2)  # w2 scale
        fpack[:, 2 * KI + NT:] = sg_t
        ipack = np.concatenate([idx_list[m], sidx], axis=1)
        in_maps.append({
            "x_bf": _bf16(x[sl]),
            "w1": q1,
            "w3": q3,
            "w2": q2,
            "sw1s": _bf16(sw1[:, ss]),
            "sw3s": _bf16(sw3[:, ss]),
            "sw2s": _bf16(sw2[ss, :]),
            "fpack": fpack,
            "ipack": np.ascontiguousarray(ipack),
        })
    return in_maps


def kernel(**inputs):
    global LAST_RESULT
    from concourse.bass_utils import run_bass_kernel_spmd

    skip = tuple(s for s in os.environ.get("KERNEL_SKIP", "").split(",") if s)
    key = ("nc", skip)
    if key not in _RUNNER:
        _RUNNER[key] = build_program(skip=skip)
    nc = _RUNNER[key]

    in_maps = _host_prep(inputs)
    trace = os.environ.get("KERNEL_TRACE", "0") == "1"
    import time
    t0 = time.perf_counter_ns()
    res = run_bass_kernel_spmd(nc, in_maps, list(range(NCORES)), trace=trace)
    global LAST_WALL_NS
    LAST_WALL_NS = time.perf_counter_ns() - t0
    LAST_RESULT = res
    out = np.concatenate([res.results[m]["out"] for m in range(NCORES)], axis=0)
    return out.reshape(B, S, H).astype(np.float32)


if __name__ == "__main__":
    # smoke build
    nc = build_program()
    print("program built ok")
